# revision 15
# baseline (speedup 1.0000x reference)
"""Trainium2 Bass kernel for the Laplace-kernel feature expansion.

Reference computation (per scalar x of the [16, 64, 64, 64] input):
    phi_i  = exp(-|x - p_i|)            for 15 design points p_i
    out_j  = sum_i chol_inv[i, j] phi_i
scattered so out[b, c*15 + j, h, w] comes from x[b, c, h, w].

Distribution: pure data parallel, 2 batches per core across 8 cores.

Per-core dataflow (no collectives):
  1. x is pre-split on host into bf16 (hi, lo) pairs, laid out so one
     [128, 16384] DMA (32 KB contiguous per partition, all 16 DMA
     engines) loads the whole per-core input into SBUF once.
  2. TensorE "broadcast" matmuls with a 0/1 block matrix replicate each
     x value onto 15 partitions (8 channel groups x 15 = 120 partitions),
     reconstructing fp32 x = hi + lo in PSUM; an extra ones-row makes the
     same matmul subtract the design point p_i (p_i exact in bf16).
     The K=17 matmuls are packed 4x into the 128x128 array via
     tile_position row-tiling (4 concurrent quadrant matmuls).
  3. VectorE computes |T| in one op (sign-bit clear on an int32 view).
  4. ScalarE computes exp(-|t|) -> bf16.
  5. TensorE applies block-diag(chol_inv) -> PSUM (fp32).
  6. PSUM evicted to SBUF (split between ScalarE/VectorE), DMA to DRAM.

Spatial mapping: PE-array quadrant q = 2j+l covers, within a (b, cblock)
tile, the spatial columns 2048j + 1024h + 512l + c (h = half), so each
post-projection PSUM chunk evicts to a contiguous 1024-column span.
"""

import sys

if "/opt/trn_rl_repo" not in sys.path:
    sys.path.insert(0, "/opt/trn_rl_repo")

import numpy as np
import ml_dtypes

BF16 = ml_dtypes.bfloat16

B, C, H, W = 16, 64, 64, 64
P = H * W                # 4096 spatial positions
M_PTS = 15               # design points
G = 8                    # channel groups per tile
MROWS = G * M_PTS        # 120 partitions used
KIN = 2 * G + 1          # 17 moving rows for the broadcast matmul
NCORES = 8
BPC = B // NCORES        # batches per core (2)
CBLK = C // G            # channel-block tiles per batch (8)
NTILES = BPC * CBLK      # 16 (b, cblock) tiles per core
QCOLS = NTILES * 1024    # 16384 columns per quadrant row

# Of the 64 PSUM->SBUF evictions per core, how many go to VectorE
# (the rest go to ScalarE). Balance point of the engine cost model.
DVE_EVICT_NUM = 55
TOTAL_EVICTS = 128

_CACHED = {}


def _build_nc():
    from concourse import bacc
    import concourse.mybir as mybir
    from concourse.tile import TileContext

    dt = mybir.dt
    Act = mybir.ActivationFunctionType
    Alu = mybir.AluOpType

    nc = bacc.Bacc(
        "TRN2", target_bir_lowering=False, debug=False, num_devices=NCORES
    )
    x_full = nc.declare_dram_parameter(
        "x_full", [128, QCOLS], dt.bfloat16, isOutput=False
    )
    w4 = nc.declare_dram_parameter("w4", [128, 128], dt.bfloat16, isOutput=False)
    r_blk = nc.declare_dram_parameter(
        "r_blk", [MROWS, 128], dt.bfloat16, isOutput=False
    )
    out = nc.declare_dram_parameter(
        "out", [BPC, C * M_PTS, P], dt.bfloat16, isOutput=True
    )

    with TileContext(nc) as tc:
        with (
            tc.tile_pool(name="const", bufs=1) as cpool,
            tc.tile_pool(name="xbig", bufs=1) as xpool,
            tc.tile_pool(name="absT", bufs=3) as apool,
            tc.tile_pool(name="phi", bufs=3) as ppool,
            tc.tile_pool(name="osb", bufs=3) as opool,
            tc.tile_pool(name="psT", bufs=1, space="PSUM") as psTp,
            tc.tile_pool(name="psO", bufs=3, space="PSUM") as psOp,
            tc.tile_pool(name="psD", bufs=1, space="PSUM") as psDp,
        ):
            w4_t = cpool.tile([128, 128], dt.bfloat16)
            nc.sync.dma_start(out=w4_t[:], in_=w4[:, :])
            r_t = cpool.tile([MROWS, 128], dt.bfloat16)
            nc.sync.dma_start(out=r_t[:], in_=r_blk[:, :])

            # Whole per-core input resident in SBUF (32 KB/partition),
            # one full-width DMA so all 16 DMA engines participate.
            xbig = xpool.tile([128, QCOLS], dt.bfloat16)
            pos = 0
            for ntile_chunk in (1, 1, 2, 4, 4, 4):
                w = ntile_chunk * 1024
                nc.sync.dma_start(
                    out=xbig[:, pos : pos + w], in_=x_full[:, pos : pos + w]
                )
                pos += w

            dump = psDp.tile([128, 512], dt.float32)

            def filler():
                # dep-free matmul into a dead PSUM bank: keeps the PE
                # array active through stall windows so the HAM clock
                # gate stays at full rate. Output is never read.
                nc.tensor.matmul(
                    dump[:], w4_t[:, :], xbig[:, 0:512], start=True, stop=True
                )

            gc = 0
            for t in range(NTILES):
                b, cb = divmod(t, CBLK)
                ot = opool.tile([MROWS, P], dt.bfloat16)
                for h in range(2):
                    tchunks = [
                        psTp.tile(
                            [128, 1024],
                            dt.float32,
                            name=f"tps{j}",
                            tag=f"tps{j}",
                        )
                        for j in range(2)
                    ]
                    # 4 concurrent quadrant matmuls (row-tiled PE array)
                    for q in range(4):
                        j, l = divmod(q, 2)
                        nc.tensor.matmul(
                            tchunks[j][:, l * 512 : (l + 1) * 512],
                            w4_t[32 * q : 32 * q + KIN, :],
                            xbig[
                                32 * q : 32 * q + KIN,
                                t * 1024 + h * 512 : t * 1024 + (h + 1) * 512,
                            ],
                            start=True,
                            stop=True,
                            tile_position=(32 * q, 0),
                        )
                    filler()
                    filler()
                    for j in range(2):
                        tps = tchunks[j]
                        at = apool.tile([MROWS, 1024], dt.float32)
                        # |T| via sign-bit clear on an int32 view
                        nc.vector.tensor_scalar(
                            out=at[:].bitcast(dt.int32),
                            in0=tps[0:MROWS, :].bitcast(dt.int32),
                            scalar1=0x7FFFFFFF,
                            scalar2=None,
                            op0=Alu.bitwise_and,
                        )
                        pt = ppool.tile([MROWS, 1024], dt.bfloat16)
                        nc.scalar.activation(pt[:], at[:], Act.Exp, scale=-1.0)
                        for l in range(2):
                            ops = psOp.tile([128, 512], dt.float32)
                            nc.tensor.matmul(
                                ops[:],
                                r_t[:],
                                pt[:, l * 512 : (l + 1) * 512],
                                start=True,
                                stop=True,
                            )
                            base = 2048 * j + 1024 * h + 512 * l
                            dst = ot[:, base : base + 512]
                            if (gc * DVE_EVICT_NUM) % TOTAL_EVICTS < DVE_EVICT_NUM:
                                nc.vector.tensor_copy(out=dst, in_=ops[0:MROWS, :])
                            else:
                                nc.scalar.activation(dst, ops[0:MROWS, :], Act.Copy)
                            gc += 1
                        filler()
                nc.sync.dma_start(
                    out=out[b, cb * MROWS : (cb + 1) * MROWS, :], in_=ot[:]
                )
    nc.compile()
    return nc


def _host_prep(x, design_points, chol_inv):
    """Build the derived host-side arrays fed to the device."""
    pts = np.asarray(design_points, dtype=np.float32)
    xs = np.ascontiguousarray(np.asarray(x, dtype=np.float32)).reshape(B, C, P)
    x_hi = xs.astype(BF16)
    x_lo = (xs - x_hi.astype(np.float32)).astype(BF16)

    # spatial = 2048j + 1024h + 512l + c ; quadrant q = 2j + l
    # arr[q, r, b, cb, h, c(512)] with r = 2g + part (hi/lo), r=16 -> 1.0
    def to_quad(a):  # [B, C, P] -> [4(q), G, B, CBLK, 2(h), 512]
        a7 = a.reshape(B, CBLK, G, 2, 2, 2, 512)  # [b, cb, g, j, h, l, c]
        return a7.transpose(3, 5, 2, 0, 1, 4, 6).reshape(4, G, B, CBLK, 2, 512)

    arr = np.empty((4, KIN, B, CBLK, 2, 512), dtype=BF16)
    arr[:, 0 : 2 * G : 2] = to_quad(x_hi)
    arr[:, 1 : 2 * G : 2] = to_quad(x_lo)
    arr[:, 2 * G] = BF16(1.0)

    w17 = np.zeros((KIN, 128), dtype=np.float32)
    for g in range(G):
        w17[2 * g, 15 * g : 15 * g + 15] = 1.0
        w17[2 * g + 1, 15 * g : 15 * g + 15] = 1.0
        w17[2 * G, 15 * g : 15 * g + 15] = -pts
    w4 = np.zeros((128, 128), dtype=np.float32)
    for q in range(4):
        w4[32 * q : 32 * q + KIN] = w17
    w4 = w4.astype(BF16)

    chol = np.asarray(chol_inv, dtype=np.float32)
    r_blk = np.zeros((MROWS, 128), dtype=np.float32)
    for g in range(G):
        r_blk[15 * g : 15 * g + 15, 15 * g : 15 * g + 15] = chol
    r_blk = r_blk.astype(BF16)

    return arr, w4, r_blk


LAST_RESULT = None


def kernel(x, design_points, chol_inv):
    global LAST_RESULT
    from concourse.bass_utils import run_bass_kernel_spmd

    if "nc" not in _CACHED:
        _CACHED["nc"] = _build_nc()
    nc = _CACHED["nc"]

    arr, w4, r_blk = _host_prep(x, design_points, chol_inv)

    in_maps = []
    for core in range(NCORES):
        # per-core [4, 17, 16384] placed into a [128, 16384] buffer at
        # partition offsets 32q (rows 17..31 of each quadrant unused)
        x_q = arr[:, :, core * BPC : (core + 1) * BPC].reshape(4, KIN, QCOLS)
        x_full = np.zeros((128, QCOLS), dtype=BF16)
        for q in range(4):
            x_full[32 * q : 32 * q + KIN] = x_q[q]
        in_maps.append({"x_full": x_full, "w4": w4, "r_blk": r_blk})

    res = run_bass_kernel_spmd(nc, in_maps, core_ids=list(range(NCORES)))
    LAST_RESULT = res

    full = np.empty((B, C * M_PTS, P), dtype=np.float32)
    for core in range(NCORES):
        full[core * BPC : (core + 1) * BPC] = res.results[core]["out"]
    return full.reshape(B, C * M_PTS, H, W)


# revision 16
# speedup vs baseline: 1.2092x; 1.2092x over previous
"""Trainium2 Bass kernel for the Laplace-kernel feature expansion.

Reference computation (per scalar x of the [16, 64, 64, 64] input):
    phi_i  = exp(-|x - p_i|)            for 15 design points p_i
    out_j  = sum_i chol_inv[i, j] phi_i
scattered so out[b, c*15 + j, h, w] comes from x[b, c, h, w].

Distribution: pure data parallel, 2 batches per core across 8 cores.

Per-core dataflow (no collectives):
  1. x is pre-split on host into bf16 (hi, lo) pairs, laid out so one
     [128, 16384] DMA (32 KB contiguous per partition, all 16 DMA
     engines) loads the whole per-core input into SBUF once.
  2. TensorE "broadcast" matmuls with a 0/1 block matrix replicate each
     x value onto 15 partitions (8 channel groups x 15 = 120 partitions),
     reconstructing fp32 x = hi + lo in PSUM; an extra ones-row makes the
     same matmul subtract the design point p_i (p_i exact in bf16).
     The K=17 matmuls are packed 4x into the 128x128 array via
     tile_position row-tiling (4 concurrent quadrant matmuls).
  3. VectorE computes |T| in one op (sign-bit clear on an int32 view).
  4. ScalarE computes exp(-|t|) -> bf16.
  5. TensorE applies block-diag(chol_inv) -> PSUM (fp32).
  6. PSUM evicted to SBUF (split between ScalarE/VectorE), DMA to DRAM.

Spatial mapping: PE-array quadrant q = 2j+l covers, within a (b, cblock)
tile, the spatial columns 2048j + 1024h + 512l + c (h = half), so each
post-projection PSUM chunk evicts to a contiguous 1024-column span.
"""

import sys

if "/opt/trn_rl_repo" not in sys.path:
    sys.path.insert(0, "/opt/trn_rl_repo")

import numpy as np
import ml_dtypes

BF16 = ml_dtypes.bfloat16

B, C, H, W = 16, 64, 64, 64
P = H * W                # 4096 spatial positions
M_PTS = 15               # design points
G = 8                    # channel groups per tile
MROWS = G * M_PTS        # 120 partitions used
KIN = 2 * G + 1          # 17 moving rows for the broadcast matmul
NCORES = 8
BPC = B // NCORES        # batches per core (2)
CBLK = C // G            # channel-block tiles per batch (8)
NTILES = BPC * CBLK      # 16 (b, cblock) tiles per core
QCOLS = NTILES * 1024    # 16384 columns per quadrant row

# Of the 64 PSUM->SBUF evictions per core, how many go to VectorE
# (the rest go to ScalarE). Balance point of the engine cost model.
DVE_EVICT_NUM = 28
TOTAL_EVICTS = 64

_CACHED = {}


def _build_nc():
    from concourse import bacc
    import concourse.mybir as mybir
    from concourse.tile import TileContext

    dt = mybir.dt
    Act = mybir.ActivationFunctionType
    Alu = mybir.AluOpType

    nc = bacc.Bacc(
        "TRN2", target_bir_lowering=False, debug=False, num_devices=NCORES
    )
    x_full = nc.declare_dram_parameter(
        "x_full", [128, QCOLS], dt.bfloat16, isOutput=False
    )
    w4 = nc.declare_dram_parameter("w4", [128, 128], dt.bfloat16, isOutput=False)
    r_blk = nc.declare_dram_parameter(
        "r_blk", [MROWS, 128], dt.bfloat16, isOutput=False
    )
    out = nc.declare_dram_parameter(
        "out", [BPC, C * M_PTS, P], dt.bfloat16, isOutput=True
    )

    with TileContext(nc) as tc:
        with (
            tc.tile_pool(name="const", bufs=1) as cpool,
            tc.tile_pool(name="xbig", bufs=1) as xpool,
            tc.tile_pool(name="absT", bufs=3) as apool,
            tc.tile_pool(name="phi", bufs=3) as ppool,
            tc.tile_pool(name="osb", bufs=3) as opool,
            tc.tile_pool(name="psT", bufs=1, space="PSUM") as psTp,
            tc.tile_pool(name="psO", bufs=2, space="PSUM") as psOp,
        ):
            # Whole per-core input resident in SBUF (32 KB/partition),
            # graduated full-width DMAs so all 16 DMA engines participate
            # and the first tile's data (plus weights) arrives quickly.
            xbig = xpool.tile([128, QCOLS], dt.bfloat16)
            nc.sync.dma_start(out=xbig[:, 0:1024], in_=x_full[:, 0:1024])
            w4_t = cpool.tile([128, 128], dt.bfloat16)
            nc.sync.dma_start(out=w4_t[:], in_=w4[:, :])
            r_t = cpool.tile([MROWS, 128], dt.bfloat16)
            nc.sync.dma_start(out=r_t[:], in_=r_blk[:, :])
            pos = 1024
            for ntile_chunk in (1, 2, 4, 4, 4):
                w = ntile_chunk * 1024
                nc.sync.dma_start(
                    out=xbig[:, pos : pos + w], in_=x_full[:, pos : pos + w]
                )
                pos += w

            gc = 0
            for t in range(NTILES):
                b, cb = divmod(t, CBLK)
                ot = opool.tile([MROWS, P], dt.bfloat16)
                for h in range(2):
                    tchunks = [
                        psTp.tile(
                            [128, 1024],
                            dt.float32,
                            name=f"tps{j}",
                            tag=f"tps{j}",
                        )
                        for j in range(2)
                    ]
                    # 4 concurrent quadrant matmuls (row-tiled PE array)
                    for q in range(4):
                        j, l = divmod(q, 2)
                        nc.tensor.matmul(
                            tchunks[j][:, l * 512 : (l + 1) * 512],
                            w4_t[32 * q : 32 * q + KIN, :],
                            xbig[
                                32 * q : 32 * q + KIN,
                                t * 1024 + h * 512 : t * 1024 + (h + 1) * 512,
                            ],
                            start=True,
                            stop=True,
                            tile_position=(32 * q, 0),
                        )
                    for j in range(2):
                        tps = tchunks[j]
                        at = apool.tile([MROWS, 1024], dt.float32)
                        # |T| via sign-bit clear on an int32 view
                        nc.vector.tensor_scalar(
                            out=at[:].bitcast(dt.int32),
                            in0=tps[0:MROWS, :].bitcast(dt.int32),
                            scalar1=0x7FFFFFFF,
                            scalar2=None,
                            op0=Alu.bitwise_and,
                        )
                        pt = ppool.tile([MROWS, 1024], dt.bfloat16)
                        nc.scalar.activation(pt[:], at[:], Act.Exp, scale=-1.0)
                        ops = psOp.tile([128, 1024], dt.float32)
                        for l in range(2):
                            nc.tensor.matmul(
                                ops[:, l * 512 : (l + 1) * 512],
                                r_t[:],
                                pt[:, l * 512 : (l + 1) * 512],
                                start=True,
                                stop=True,
                            )
                        # contiguous spatial span 2048j + 1024h
                        dst = ot[:, 2048 * j + 1024 * h : 2048 * j + 1024 * h + 1024]
                        if (gc * DVE_EVICT_NUM) % TOTAL_EVICTS < DVE_EVICT_NUM:
                            nc.vector.tensor_copy(out=dst, in_=ops[0:MROWS, :])
                        else:
                            nc.scalar.activation(dst, ops[0:MROWS, :], Act.Copy)
                        gc += 1
                nc.sync.dma_start(
                    out=out[b, cb * MROWS : (cb + 1) * MROWS, :], in_=ot[:]
                )
    nc.compile()
    return nc


def _host_prep(x, design_points, chol_inv):
    """Build the derived host-side arrays fed to the device."""
    pts = np.asarray(design_points, dtype=np.float32)
    xs = np.ascontiguousarray(np.asarray(x, dtype=np.float32)).reshape(B, C, P)
    x_hi = xs.astype(BF16)
    x_lo = (xs - x_hi.astype(np.float32)).astype(BF16)

    # spatial = 2048j + 1024h + 512l + c ; quadrant q = 2j + l
    # arr[q, r, b, cb, h, c(512)] with r = 2g + part (hi/lo), r=16 -> 1.0
    def to_quad(a):  # [B, C, P] -> [4(q), G, B, CBLK, 2(h), 512]
        a7 = a.reshape(B, CBLK, G, 2, 2, 2, 512)  # [b, cb, g, j, h, l, c]
        return a7.transpose(3, 5, 2, 0, 1, 4, 6).reshape(4, G, B, CBLK, 2, 512)

    arr = np.empty((4, KIN, B, CBLK, 2, 512), dtype=BF16)
    arr[:, 0 : 2 * G : 2] = to_quad(x_hi)
    arr[:, 1 : 2 * G : 2] = to_quad(x_lo)
    arr[:, 2 * G] = BF16(1.0)

    w17 = np.zeros((KIN, 128), dtype=np.float32)
    for g in range(G):
        w17[2 * g, 15 * g : 15 * g + 15] = 1.0
        w17[2 * g + 1, 15 * g : 15 * g + 15] = 1.0
        w17[2 * G, 15 * g : 15 * g + 15] = -pts
    w4 = np.zeros((128, 128), dtype=np.float32)
    for q in range(4):
        w4[32 * q : 32 * q + KIN] = w17
    w4 = w4.astype(BF16)

    chol = np.asarray(chol_inv, dtype=np.float32)
    r_blk = np.zeros((MROWS, 128), dtype=np.float32)
    for g in range(G):
        r_blk[15 * g : 15 * g + 15, 15 * g : 15 * g + 15] = chol
    r_blk = r_blk.astype(BF16)

    return arr, w4, r_blk


LAST_RESULT = None


def kernel(x, design_points, chol_inv):
    global LAST_RESULT
    from concourse.bass_utils import run_bass_kernel_spmd

    if "nc" not in _CACHED:
        _CACHED["nc"] = _build_nc()
    nc = _CACHED["nc"]

    arr, w4, r_blk = _host_prep(x, design_points, chol_inv)

    in_maps = []
    for core in range(NCORES):
        # per-core [4, 17, 16384] placed into a [128, 16384] buffer at
        # partition offsets 32q (rows 17..31 of each quadrant unused)
        x_q = arr[:, :, core * BPC : (core + 1) * BPC].reshape(4, KIN, QCOLS)
        x_full = np.zeros((128, QCOLS), dtype=BF16)
        for q in range(4):
            x_full[32 * q : 32 * q + KIN] = x_q[q]
        in_maps.append({"x_full": x_full, "w4": w4, "r_blk": r_blk})

    res = run_bass_kernel_spmd(nc, in_maps, core_ids=list(range(NCORES)))
    LAST_RESULT = res

    full = np.empty((B, C * M_PTS, P), dtype=np.float32)
    for core in range(NCORES):
        full[core * BPC : (core + 1) * BPC] = res.results[core]["out"]
    return full.reshape(B, C * M_PTS, H, W)
